# revision 69
# baseline (speedup 1.0000x reference)
"""Trainium2 Bass kernel for CantorMultiheadFusion.

Reference math:
    h      = x @ W_in^T                        # [B,S,D]
    d[s,k] = distances[s, routes[s,k]]
    w      = softmax(-d, axis=-1)              # [S,K]
    fused  = sum_k w[s,k] * h[:, routes[s,k]]  # [B,S,D]  (head reshape is a no-op)
    out    = fused @ W_out^T + b_out + x

The fusion weights are shared across the feature dim, so the gather commutes
with both projections:
    out[s] = sum_j A[s,j] * (x[j] @ WcT) + b_out + x[s]
where A[s,j] = C[s,j] * exp(-distances[s,j]) / denom(s),
      C[s,j] = #{k : routes[s,k] == j}   (integer multiplicity),
      WcT[d,i] = sum_e W_out[i,e] W_in[e,d],
      denom(s) = sum_j C[s,j] exp(-dist[s,j]).
C is built host-side from the int32 routes tensor (index marshalling); all
float math on actual inputs (exp, reciprocal, normalization, matmuls,
residual) runs on device.

Sharding: sequence-parallel over S across 8 cores (SLOC=256 rows each); x is
replicated since the A@x contraction needs all S rows.

Design notes (driven by the TimelineSim cost model):
  * DMA: serial device @360GB/s, 2x penalty under 512B contiguous runs,
    ~630ns HWDGE per DMA, 900ns completion-sem. Everything is host pre-laid
    in final SBUF order; fp8 x/dist/C/W, bf16 residual; ~10 large DMAs.
  * PE: fp8 DoubleRow matmuls everywhere (2 contraction rows/partition, 0.5
    cycles/row). The contraction runs in j-PAIRS (2x128 rows); per core,
    pair slot 0 is its own S-slice, recovered on-chip from the bf16
    residual stream (never DMA'd in fp8). Host ships per-core pair order
    [local, remotes...] consistently for x and dist/C. PSUM accumulation
    start/stop ride slots 1/7; slot 0 accumulates mid-sequence since its
    operands (converted x, last dc chunk) land later.
  * Epilogue: denominators come from tiny DoubleRow ones-matmuls over
    numer (pdT, per-s partitions). The residual (+bias) is preloaded into
    stepB's PSUM banks scaled by denom via a diagonal matmul
    (diagden = identity * denom[s]), so po = den*res + proj and the whole
    epilogue is ONE per-partition rd-scaled PSUM->SBUF copy — DVE's
    tensor_scalar for sc0, ACT's activation-scale for sc1 — then the out
    DMA. Exact math: (den*res + proj) * (1/den) = res + proj/den.
  * b0's x streams fully before b1's (each in four DMA groups so stepA
    tracks the stream), so b0's whole tail overlaps b1's stream and only
    b1's short chain trails the last DMA byte.

Measured: ~9e-3 max rel err vs fp32 reference (gate 2e-2), hardware-checked.
"""

import os
import sys

import numpy as np

for _p in ("/opt/trn_rl_repo",):
    if os.path.isdir(_p) and _p not in sys.path:
        sys.path.insert(0, _p)


# Some container snapshots lack antenv.axon_hooks (the axon NTFF profile
# hook); stub it so run_bass_kernel_spmd(trace=True) degrades gracefully.
def _ensure_axon_hooks_stub():
    import types
    try:
        import antenv.axon_hooks  # noqa: F401
    except ModuleNotFoundError:
        try:
            import antenv
        except ModuleNotFoundError:
            return
        _stub = types.ModuleType("antenv.axon_hooks")
        _stub.get_axon_ntff_profile_hook = lambda: None
        sys.modules["antenv.axon_hooks"] = _stub
        antenv.axon_hooks = _stub


_ensure_axon_hooks_stub()

B, S, D, K = 2, 2048, 512, 64
N_CORES = 8
SLOC = S // N_CORES          # 256 sequence rows per core
NP = S // 256                # 8 contraction pairs (2x128 rows each)
NREM = NP - 1                # remote pairs DMA'd in fp8
NSC = SLOC // 128            # 2 seq chunks per core
NE = D // 128                # 4 feature chunks
PB = 1024                    # elems per pair per batch in x layout (2*512)

# dc memory order: [p1 | p2 .. p7 | p0], each pair 1024 (d|C). DMA chunks
# (byte cols): biggest first so the head has no HWDGE bubble; pair 0
# (local) last — its numer is only needed mid-schedule.
DCCHUNKS = [(0, 4 * 1024), (4 * 1024, 7 * 1024), (7 * 1024, 8 * 1024)]
EXPORDER = [1, 2, 3, 4, 5, 6, 0, 7]       # numer pipeline order
# stepA accumulation order per batch: slot 0 (local, converted x + last dc
# chunk) slides in as late as its operands allow without delaying the
# stop-slot; start/stop ride the first/last entries.
AORDER0 = [1, 2, 3, 4, 5, 6, 0, 7]
AORDER1 = [0, 1, 2, 3, 4, 5, 6, 7]

_CACHE = {}
LAST_RESULTS = None


def _dcoff(k):
    """Column offset of pair slot k's (d|C) block in the dc tensor."""
    return (7 if k == 0 else k - 1) * 1024


def _build_nc(with_bias=True):
    import concourse.bacc as bacc
    import concourse.mybir as mybir
    import concourse.tile as tile

    F32 = mybir.dt.float32
    BF16 = mybir.dt.bfloat16
    F8 = mybir.dt.float8e4
    DR = mybir.MatmulPerfMode.DoubleRow

    nc = bacc.Bacc("TRN2", target_bir_lowering=False, debug=False, num_devices=1)

    # [p, pair, {dist,C}, i, s] fp8
    dc_d = nc.dram_tensor("dc", [128, NP * 1024], F8,
                          kind="ExternalInput").ap()
    # [p, b, remote, i, e] fp8
    x_d = nc.dram_tensor("x", [128, B * NREM * PB], F8, kind="ExternalInput").ap()
    # [p, b, sc, e] bf16 residual + [p, 128] identity
    xr_d = nc.dram_tensor("xr", [128, B * NSC * D + 128], BF16,
                          kind="ExternalInput").ap()
    # [p, {win,woutT}, apair, i, e] fp8
    w_d = nc.dram_tensor("w", [128, 2 * 2 * PB], F8, kind="ExternalInput").ap()
    bo_d = nc.dram_tensor("b_out", [1, D], F32, kind="ExternalInput").ap()
    out_d = nc.dram_tensor("out", [B, SLOC, D], F32, kind="ExternalOutput").ap()

    with tile.TileContext(nc) as tc:
        with (
            tc.tile_pool(name="big", bufs=1) as big,
            tc.tile_pool(name="pa", bufs=5, space="PSUM") as pa,
            tc.tile_pool(name="pden", bufs=1, space="PSUM") as pdenp,
            tc.tile_pool(name="ptr", bufs=2, space="PSUM") as ptr,
        ):
            # ---- persistent SBUF ----
            dcbuf = big.tile([128, NP * 1024], F8)
            xbuf = big.tile([128, B * NP * PB], F8)       # [p,(b,k,i,e)]
            numer = big.tile([128, NP * 512], F8)         # C*exp(-d)
            wbuf = big.tile([128, 2 * 2 * PB], F8)        # [p,(w,ap,i,e)]
            wcT = big.tile([128, 2 * PB], F8)             # [p,(dp,i,iout)]
            xres_sb = big.tile([128, B * NSC * D + 128], BF16)
            tT = big.tile([128, B * PB], F8)              # [p,(b,dp,i,s)]
            outbuf = big.tile([128, B * NSC * D], F32)
            ones2 = big.tile([128, 2], F8)
            rdT = big.tile([128, NSC], F32)
            denT = big.tile([128, NSC], F32)
            diagden = big.tile([128, NSC * 128], BF16)    # den[s]-scaled identity
            if with_bias:
                bias_sb = big.tile([1, D], F32)
                bias_bc = big.tile([128, D], BF16)
                ones_rf = big.tile([1, 128], F32)

            nc.vector.memset(ones2[:], 1.0)
            ones2_3 = ones2[:].rearrange("p (i o) -> p i o", i=2)

            def dsl(k):
                return dcbuf[:, _dcoff(k):_dcoff(k) + 512]

            def csl(k):
                return dcbuf[:, _dcoff(k) + 512:_dcoff(k) + 1024]

            def nsl(k):
                return numer[:, k * 512:(k + 1) * 512]

            def nsl3(k):
                return nsl(k).rearrange("p (i s) -> p i s", i=2)

            def xap(b, k, ec):
                sl = xbuf[:, (b * NP + k) * PB:(b * NP + k + 1) * PB]
                return sl.rearrange("p (i e) -> p i e", i=2)[:, :, ec * 128:(ec + 1) * 128]

            def win_ap(ap_, ec):
                sl = wbuf[:, ap_ * PB:(ap_ + 1) * PB]
                return sl.rearrange("p (i e) -> p i e", i=2)[:, :, ec * 128:(ec + 1) * 128]

            def wout_ap(ap_):
                sl = wbuf[:, (2 + ap_) * PB:(3 + ap_) * PB]
                return sl.rearrange("p (i e) -> p i e", i=2)

            def wcT3(dp):
                return wcT[:, dp * PB:(dp + 1) * PB].rearrange("p (i e) -> p i e", i=2)

            def tT3(b, dp, sc):
                sl = tT[:, (b * 2 + dp) * 512:(b * 2 + dp + 1) * 512]
                return sl.rearrange("p (i s) -> p i s", i=2)[:, :, sc * 128:(sc + 1) * 128]

            def chunk(buf, b, sc):
                o = (b * NSC + sc) * D
                return buf[:, o:o + D]

            ident = xres_sb[:, B * NSC * D:B * NSC * D + 128]

            # stepA PSUM: bank (b,dp) holds ec=2dp (cols :256) and ec=2dp+1
            # (cols 256:). One start=True per bank (first AORDER slot, even
            # ec): start clears the whole bank's has_written bits so the
            # odd-ec group overwrites on first write.
            pts = {(b, dp): pa.tile([128, 512], F32, name=f"pts{b}_{dp}", tag="acc")
                   for b in range(B) for dp in range(NE // 2)}

            def pta(b, ec):
                return pts[(b, ec // 2)][:, (ec % 2) * 256:(ec % 2 + 1) * 256]

            # ================= DMA input stream (sync queue, in order) ====
            x_d4 = x_d.rearrange("p (b r f) -> p b r f", b=B, r=NREM)
            xbuf4 = xbuf[:].rearrange("p (b k f) -> p b k f", b=B, k=NP)

            def xdma(b, r0, r1):
                nc.sync.dma_start(out=xbuf4[:, b, 1 + r0:1 + r1, :],
                                  in_=x_d4[:, b, r0:r1, :])

            for (c0, c1) in DCCHUNKS:
                nc.sync.dma_start(out=dcbuf[:, c0:c1], in_=dc_d[:, c0:c1])
            nc.sync.dma_start(out=wbuf[:], in_=w_d[:, :])
            nc.sync.dma_start(out=xres_sb[:], in_=xr_d[:, :])
            if with_bias:
                nc.scalar.dma_start(out=bias_sb[:1, :], in_=bo_d[:, :])
            xdma(0, 0, 2)
            xdma(0, 2, 4)
            xdma(0, 4, 6)
            xdma(0, 6, 7)
            xdma(1, 0, 2)
            xdma(1, 2, 4)
            xdma(1, 4, 6)
            xdma(1, 6, 7)

            # ================= numer pipeline (ACT exp + DVE mult) ========
            for k in EXPORDER:
                nc.scalar.activation(nsl(k), dsl(k),
                                     mybir.ActivationFunctionType.Exp, scale=-1.0)
                nc.vector.tensor_mul(nsl(k), nsl(k), csl(k))

            # local pair x: bf16 residual rows -> fp8 stepA operand (slot 0).
            # Both on Pool (slow but idle; DVE must finish the mults early).
            nc.gpsimd.tensor_copy(xbuf[:, 0:PB], xres_sb[:, 0:PB])
            nc.gpsimd.tensor_copy(xbuf[:, NP * PB:NP * PB + PB],
                                  xres_sb[:, PB:2 * PB])

            # denominators: DR ones-matmul per (slot, sc), numer-arrival
            # order. denT (for the residual preload) and rdT (for the
            # scaled outcopy) are only needed in the tail (~12us+).
            # full-bank shape so pden has one size class; cols 0:NSC used
            pdT = pdenp.tile([128, 512], F32, name="pdT", tag="acc")
            for ki, k in enumerate(EXPORDER):
                for sc in range(NSC):
                    nc.tensor.matmul(
                        pdT[:, sc:sc + 1],
                        lhsT=nsl3(k)[:, :, sc * 128:(sc + 1) * 128],
                        rhs=ones2_3,
                        start=(ki == 0 and sc == 0),
                        stop=(ki == NP - 1 and sc == NSC - 1),
                        perf_mode=DR,
                        skip_group_check=True)
            nc.vector.reciprocal(rdT[:], pdT[:, 0:NSC])
            nc.vector.tensor_copy(denT[:], pdT[:, 0:NSC])
            # diagden[sc] = identity * den[s]: lhsT of the residual preload,
            # so po accumulates den*res + proj and the final per-partition
            # rd scale yields res + proj/denom exactly.
            with nc.allow_low_precision(reason="den in bf16: 0.4% of the "
                                        "residual, ~2e-3 on the output"):
                for sc in range(NSC):
                    nc.vector.tensor_scalar_mul(
                        diagden[:, sc * 128:(sc + 1) * 128], ident,
                        denT[:, sc:sc + 1])

            if with_bias:
                nc.vector.memset(ones_rf[:], 1.0)
                pb = ptr.tile([128, D], F32, name="pb", tag="tr")
                nc.tensor.matmul(pb[:], lhsT=ones_rf[:1, :], rhs=bias_sb[:1, :],
                                 start=True, stop=True)
                with nc.allow_low_precision(reason="bias broadcast to bf16 "
                                            "for the preload matmul rhs"):
                    nc.vector.tensor_copy(bias_bc[:], pb[:])

            # ================= PE program =================================
            def stepa(k, b):
                aorder = AORDER0 if b == 0 else AORDER1
                first = (k == aorder[0])
                last = (k == aorder[-1])
                for ec in range(NE):
                    nc.tensor.matmul(
                        pta(b, ec),
                        lhsT=xap(b, k, ec),
                        rhs=nsl3(k),
                        start=(first and ec % 2 == 0),
                        stop=(last and ec % 2 == 1),
                        perf_mode=DR,
                        skip_group_check=True)

            # WcT[d,i] = sum_a W_in[a,d] * W_outT[a,i]
            for ec in range(NE):
                pw = ptr.tile([128, D], F32, name=f"pw{ec}", tag="tr")
                for ap_ in range(2):
                    nc.tensor.matmul(pw[:], lhsT=win_ap(ap_, ec), rhs=wout_ap(ap_),
                                     start=(ap_ == 0), stop=(ap_ == 1),
                                     perf_mode=DR)
                nc.scalar.copy(wcT[:, ec * D:(ec + 1) * D], pw[:])

            # ================= tail: copies, stepB, outcopy, out ==========
            def tail(b):
                # PSUM->SBUF copies (dp0 DVE, dp1 ACT; b1 split per-sc so
                # its stepB sc0 starts before the sc1 halves land); stepB po
                # banks come from pa spares so nothing waits on a drain; the
                # den-scaled residual (+bias) is pre-accumulated into po via
                # the diagden matmul, so the epilogue is one per-partition
                # rd-scaled copy split across DVE/ACT.
                if b == 0:
                    nc.vector.tensor_copy(tT[:, (b * 2) * 512:(b * 2 + 1) * 512],
                                          pts[(b, 0)][:])
                    nc.scalar.copy(tT[:, (b * 2 + 1) * 512:(b * 2 + 2) * 512],
                                   pts[(b, 1)][:])
                else:
                    for scc in range(NSC):
                        for dp in range(2):
                            src3 = pts[(b, dp)][:].rearrange(
                                "p (i s) -> p i s", i=2)[:, :, scc * 128:(scc + 1) * 128]
                            dst3 = tT[:, (b * 2 + dp) * 512:(b * 2 + dp + 1) * 512] \
                                .rearrange("p (i s) -> p i s", i=2)[:, :, scc * 128:(scc + 1) * 128]
                            if dp == 0:
                                nc.vector.tensor_copy(dst3, src3)
                            else:
                                nc.scalar.copy(dst3, src3)
                for sc in range(NSC):
                    dds = diagden[:, sc * 128:(sc + 1) * 128]
                    po = pa.tile([128, D], F32, name=f"po{b}_{sc}", tag="acc")
                    nc.tensor.matmul(po[:], lhsT=dds, rhs=chunk(xres_sb, b, sc),
                                     start=True, stop=False,
                                     skip_group_check=True)
                    if with_bias:
                        nc.tensor.matmul(po[:], lhsT=dds, rhs=bias_bc[:],
                                         start=False, stop=False,
                                         skip_group_check=True)
                    for dp in range(2):
                        nc.tensor.matmul(po[:], lhsT=tT3(b, dp, sc), rhs=wcT3(dp),
                                         start=False, stop=(dp == 1),
                                         perf_mode=DR,
                                         skip_group_check=True)
                    if sc == 0:
                        nc.vector.tensor_scalar_mul(chunk(outbuf, b, sc), po[:],
                                                    rdT[:, sc:sc + 1])
                    else:
                        nc.scalar.activation(chunk(outbuf, b, sc), po[:],
                                             mybir.ActivationFunctionType.Copy,
                                             scale=rdT[:, sc:sc + 1])
                    nc.sync.dma_start(
                        out=out_d[b, sc * 128:(sc + 1) * 128, :],
                        in_=chunk(outbuf, b, sc))

            for k in AORDER0:
                stepa(k, 0)
            tail(0)
            for k in AORDER1:
                stepa(k, 1)
            tail(1)

    nc.compile()
    return nc


def _get_nc(with_bias=True):
    key = ("nc", with_bias)
    if key not in _CACHE:
        _CACHE[key] = _build_nc(with_bias)
    return _CACHE[key]


def prep_in_maps(x, routes, distances, W_in, W_out, b_out):
    """Host-side sharding/marshalling: per-core input dicts.

    Pure index marshalling + dtype casts: transposes into the on-chip tile
    layouts, fp8/bf16 casts, the routes-derived count matrix and gathered-
    distance table, and a constant identity block. No float arithmetic on
    input values happens here.
    """
    import ml_dtypes
    import concourse.mybir as mybir

    bf16 = ml_dtypes.bfloat16
    f8 = mybir.dt.np(mybir.dt.float8e4)
    x = np.ascontiguousarray(np.asarray(x, dtype=np.float32))
    routes = np.asarray(routes, dtype=np.int32)
    distances = np.ascontiguousarray(np.asarray(distances, dtype=np.float32))
    W_in = np.asarray(W_in, dtype=np.float32)
    W_out = np.asarray(W_out, dtype=np.float32)
    b_out = np.ascontiguousarray(
        np.asarray(b_out, dtype=np.float32)).reshape(1, D)

    # Count matrix C[j, s] = multiplicity of j in routes[s, :] (int32-derived)
    flat = routes.astype(np.int64).ravel() * S + \
        np.repeat(np.arange(S, dtype=np.int64), K)
    countsT = np.bincount(flat, minlength=S * S).reshape(S, S)  # [j, s]

    dt_r = np.ascontiguousarray(distances.T).astype(f8).reshape(NP, 2, 128, S)
    c_r = countsT.astype(np.float32).astype(f8).reshape(NP, 2, 128, S)

    # x in the stepA tile layout: [p, b, pair, i, e] fp8
    x_f8 = x.astype(f8).reshape(B, NP, 2, 128, D).transpose(3, 0, 1, 2, 4)

    # W in the Wc layout: [p, {win,woutT}, apair, i, e] fp8
    win_p = W_in.reshape(2, 2, 128, D).transpose(2, 0, 1, 3)
    wout_p = np.ascontiguousarray(W_out.T).reshape(2, 2, 128, D).transpose(2, 0, 1, 3)
    w_pre = np.ascontiguousarray(
        np.stack([win_p, wout_p], axis=1).astype(f8)).reshape(128, 2 * 2 * PB)

    ident = np.eye(128, dtype=bf16)

    in_maps = []
    for c in range(N_CORES):
        order = [c] + [p for p in range(NP) if p != c]
        ssl = slice(c * SLOC, (c + 1) * SLOC)
        # dc: [p, slots 1..7,0 of {d,C}] -> [128, 8192]
        dcd = dt_r[order][:, :, :, ssl].transpose(2, 0, 1, 3)   # [128,8,2,256]
        dcc = c_r[order][:, :, :, ssl].transpose(2, 0, 1, 3)
        pairs = np.stack([dcd, dcc], axis=2)                    # [128,8,2,2,256]
        pairs = pairs[:, [1, 2, 3, 4, 5, 6, 7, 0]]              # memory order
        dc = pairs.reshape(128, NP * 1024)
        # x: remote pairs only, [p, b, r, i, e] -> [128, B*7*1024]
        xc = np.ascontiguousarray(
            x_f8[:, :, order[1:], :, :]).reshape(128, B * NREM * PB)
        # xres: [p, b, sc, e] bf16 + identity block
        xr = np.concatenate([
            x[:, ssl, :].reshape(B, NSC, 128, D).transpose(2, 0, 1, 3)
            .astype(bf16).reshape(128, B * NSC * D),
            ident], axis=1)
        in_maps.append({
            "dc": np.ascontiguousarray(dc),
            "x": xc,
            "xr": np.ascontiguousarray(xr),
            "w": w_pre,
            "b_out": b_out,
        })
    return in_maps


def kernel(x, routes, distances, W_in, W_out, b_out):
    global LAST_RESULTS
    from concourse import bass_utils

    in_maps = prep_in_maps(x, routes, distances, W_in, W_out, b_out)
    with_bias = bool(np.any(np.asarray(b_out)))
    nc = _get_nc(with_bias)
    _CACHE["last_nc"] = nc
    res = bass_utils.run_bass_kernel_spmd(nc, in_maps, core_ids=list(range(N_CORES)))
    LAST_RESULTS = res
    out = np.concatenate([res.results[c]["out"] for c in range(N_CORES)], axis=1)
    return out


if __name__ == "__main__":
    rng = np.random.default_rng(0)
    inputs = {
        "x": rng.standard_normal((B, S, D), dtype=np.float32),
        "routes": rng.integers(0, S, (S, K)).astype(np.int32),
        "distances": rng.random((S, S), dtype=np.float32),
        "W_in": (rng.standard_normal((D, D), dtype=np.float32) / np.sqrt(D)).astype(np.float32),
        "W_out": (rng.standard_normal((D, D), dtype=np.float32) / np.sqrt(D)).astype(np.float32),
        "b_out": np.zeros(D, dtype=np.float32),
    }
    out = kernel(**inputs)
    print("out", out.shape, out.dtype)
